# revision 16
# baseline (speedup 1.0000x reference)
"""GQA forward on 8 Trainium2 NeuronCores — v4 (fused pipeline, lean DMA).

Sharding: core c -> batch b=c//4, kv-head pair p=c%4 (kv heads {2p,2p+1},
q heads 8p..8p+7). Each core computes a partial [T,E] output (its heads'
contribution through Wo rows); host sums the 4 partials per batch + bo.

v4 over v3: DMA instruction economy. v3's 80 per-head dma transposes cost
~1.2us of trigger descriptor-gen EACH on the issuing engine and their sem
waits head-of-line blocked the ScalarE exp queue, starving the PE for
~46us. v4 merges: one [128,640] xbar transpose per t-chunk into a packed
QTall tile (4 Q head-pair blocks + K block via 3D out AP) on the Act
ring; per-kc merged weight loads; 3-stage xt loads; single rope/wo loads;
one [128,2048] store per t-chunk. ScalarE carries only exp + 16 transpose
triggers; SP carries loads/stores.

Other structure (from v3):
- Stage A (QKV proj) software-pipelined WITH attention: projection chains
  for block k+1 and out-proj for k-1 interleave into attention block k.
- Q heads re-paired across kv groups (host Wq/Wo reorder) so the packed
  S matmul pair reads K^T from one tile without duplication.
- exp fused per chunk via strided [128,2,w] pair view; paired causal mask
  mul against a duplicated mask tile.
- Z replicated into psOT rows 64:128 by the 64-wide ones block in V;
  finalize = reciprocal + PSUM-sourced multiply into yT.
- PSUM: psS 2x[128,1024] + shared tag-"b" pool (psOT pairs / stage-C psF)
  + psQ + psKV = 8 banks.
"""
import sys
import numpy as np

sys.path.insert(0, "/opt/trn_rl_repo")

import ml_dtypes

BF16 = ml_dtypes.bfloat16

B, T, E = 2, 2048, 2048
HQ, HKV = 32, 8
D = 64
NT = T // 128          # 16 t-chunks
KC = 17                # augmented contraction chunks (2048 + bias row)
KAUG = KC * 128
NWARM = 200

_cache = {}


def _build_program():
    import concourse.bass as bass
    import concourse.tile as tile
    import concourse.mybir as mybir
    from concourse import bacc

    fp32 = mybir.dt.float32
    bf16 = mybir.dt.bfloat16
    MUL = mybir.AluOpType.mult
    ADD = mybir.AluOpType.add
    SUB = mybir.AluOpType.subtract
    EXP = mybir.ActivationFunctionType.Exp

    nc = bacc.Bacc("TRN2", target_bir_lowering=False, debug=False)

    xt_d = nc.dram_tensor("xt", [KAUG, T], bf16, kind="ExternalInput").ap()
    wqkv_d = nc.dram_tensor("wqkv", [KAUG, 768], bf16, kind="ExternalInput").ap()
    wo2_d = nc.dram_tensor("wo2", [128, 4 * T], bf16, kind="ExternalInput").ap()
    rope2_d = nc.dram_tensor("rope2", [128, NT * 512], bf16, kind="ExternalInput").ap()
    mask2_d = nc.dram_tensor("mask2", [128, 256], bf16, kind="ExternalInput").ap()
    iden_d = nc.dram_tensor("iden", [128, 128], bf16, kind="ExternalInput").ap()
    out_d = nc.dram_tensor("out", [T, E], bf16, kind="ExternalOutput").ap()

    def hv(ap, H, off, w):
        return ap.rearrange("p (h d) -> p h d", h=H)[:, :, off:off + w]

    with tile.TileContext(nc) as tc:
        with (
            tc.tile_pool(name="persist", bufs=1) as pp,
            tc.tile_pool(name="wpool", bufs=1) as wp,
            tc.tile_pool(name="xpool", bufs=1) as xp,
            tc.tile_pool(name="tmp", bufs=2) as tp,
            tc.tile_pool(name="qk", bufs=3) as qkp,
            tc.tile_pool(name="sa", bufs=3) as sap,
            tc.tile_pool(name="fin", bufs=2) as finp,
            tc.tile_pool(name="osb", bufs=2) as osbp,
            tc.tile_pool(name="psQ", bufs=1, space="PSUM") as psQp,
            tc.tile_pool(name="psKV", bufs=1, space="PSUM") as psKVp,
            tc.tile_pool(name="psS", bufs=2, space="PSUM") as psSp,
            tc.tile_pool(name="psB", bufs=2, space="PSUM") as psBp,
        ):
            iden = pp.tile([128, 128], bf16)
            nc.sync.dma_start(iden[:], iden_d[:])
            mask2 = pp.tile([128, 256], bf16)
            nc.sync.dma_start(mask2[:], mask2_d[:])
            rtsAll = xp.tile([128, NT * 512], bf16, tag="rts", name="rtsAll")
            nc.sync.dma_start(rtsAll[:, 0:2048], rope2_d[:, 0:2048])

            # per-isc-block transpose outputs: [128, 5*512]; within a block,
            # g=0..3 -> Q^T head pair g (512 cols each), block 4 -> K^T.
            # Separate tiles per isc block so the dep tracker's bounding-box
            # WAW does not chain transposes across blocks.
            QTb = [pp.tile([128, 5 * 512], bf16, tag=f"QTb{b}", name=f"QTb{b}")
                   for b in range(4)]
            yT = [pp.tile([128, T], bf16, tag=f"yT{g}", name=f"yT{g}") for g in range(4)]
            Vs = [pp.tile([128, 256], bf16, tag=f"V{j}", name=f"V{j}") for j in range(NT)]
            wqkvAll = wp.tile([128, KC * 768], bf16, tag="wqkv", name="wqkvAll")
            wosAll = wp.tile([128, 4 * T], bf16, tag="wos", name="wosAll")
            xtAll = xp.tile([128, KC * T], bf16, tag="xtAll", name="xtAll")

            # ---- prologue loads (SP ring) as a handful of mega-DMAs with
            # 3D APs: few DMA instructions keep the sem-slot-reuse guards on
            # later transposes/stores pointing at long-completed transfers.
            def xt_mega(kcs, cs):
                ov = xtAll[:].rearrange("p (kc t) -> p kc t", kc=KC)[:, kcs, cs]
                iv = xt_d[:, cs].rearrange("(kc p) t -> p kc t", p=128)[:, kcs, :]
                nc.sync.dma_start(ov, iv)

            def wqkv_mega(kcs):
                ov = wqkvAll[:].rearrange("p (kc w) -> p kc w", kc=KC)[:, kcs, :]
                iv = wqkv_d[:].rearrange("(kc p) w -> p kc w", p=128)[:, kcs, :]
                nc.sync.dma_start(ov, iv)

            wqkv_mega(slice(0, 9))
            xt_mega(slice(0, 9), slice(0, 512))
            wqkv_mega(slice(9, KC))
            xt_mega(slice(9, KC), slice(0, 512))
            xt_mega(slice(0, 9), slice(512, 1024))
            xt_mega(slice(9, KC), slice(512, 1024))
            nc.sync.dma_start(rtsAll[:, 2048:8192], rope2_d[:, 2048:8192])
            nc.sync.dma_start(wosAll[:], wo2_d[:])
            xt_mega(slice(0, 9), slice(1024, 2048))
            xt_mega(slice(9, KC), slice(1024, 2048))

            # pad the DMA sem pool (9 slots, shared across rings): ~12 tiny
            # Act-ring DMAs so later transposes' sem-reuse guards reference
            # these instead of the slow HBM mega-loads
            padt = xp.tile([1, 768], bf16, tag="padt", name="padt")
            for i in range(12):
                nc.scalar.dma_start(padt[0:1, i * 64:(i + 1) * 64],
                                    iden_d[0:1, 0:64])

            for j in range(NT):
                nc.gpsimd.memset(Vs[j][:, 64:128], 1.0)
                nc.gpsimd.memset(Vs[j][:, 192:256], 1.0)

            nbias = pp.tile([128, 1], fp32, tag="nbias", name="nbias")
            nc.gpsimd.memset(nbias[:], -4.0)

            # PE warm-up riding the initial DMA gate (uses the psQ bank)
            psW = psQp.tile([128, 512], fp32, tag="psQ", name="psW")
            for _ in range(NWARM):
                nc.tensor.matmul(psW[:, 0:128], iden[:], iden[:], start=True,
                                 stop=True, skip_group_check=True)

            # ---------------- stage A generator --------------------------
            def gen_A(blk):
                def pump():
                    pass

                for tq in range(4):
                    t_i = 4 * blk + tq
                    ts = slice(t_i * 128, (t_i + 1) * 128)
                    xsl = slice(blk * 512 + tq * 128, blk * 512 + (tq + 1) * 128)
                    psQ = psQp.tile([128, 512], fp32, tag="psQ", name="psQ")
                    for kc in range(KC):
                        nc.tensor.matmul(psQ[:], xtAll[:, kc * T + xsl.start:kc * T + xsl.stop],
                                         wqkvAll[:, kc * 768:kc * 768 + 512],
                                         start=(kc == 0), stop=(kc == KC - 1))
                    pump()
                    yield
                    psKV = psKVp.tile([128, 256], fp32, tag="psKV", name="psKV")
                    for kc in range(KC):
                        nc.tensor.matmul(psKV[:], xtAll[:, kc * T + xsl.start:kc * T + xsl.stop],
                                         wqkvAll[:, kc * 768 + 512:kc * 768 + 768],
                                         start=(kc == 0), stop=(kc == KC - 1))
                    # casts + V copy (DVE; also frees psQ/psKV for next tc)
                    Qb = tp.tile([128, 512], bf16, tag="Qb")
                    nc.vector.tensor_copy(Qb[:], psQ[:])
                    Kb = tp.tile([128, 128], bf16, tag="Kb")
                    nc.vector.tensor_copy(Kb[:], psKV[:, 0:128])
                    vo = Vs[t_i][:].rearrange("p (kv c) -> p kv c", kv=2)[:, :, 0:64]
                    vi = psKV[:, 128:256].rearrange("p (kv c) -> p kv c", kv=2)
                    nc.vector.tensor_copy(vo, vi)

                    rt = rtsAll[:, t_i * 512:(t_i + 1) * 512]
                    QKsb = qkp.tile([128, 640], bf16, tag="QKsb")
                    # RoPE Q (DVE bf16): y1' = y1*c - y2*s ; y2' = y2*c + y1*s
                    q1 = hv(Qb[:], 8, 0, 32)
                    q2 = hv(Qb[:], 8, 32, 32)
                    c8v = hv(rt[:, 0:256], 8, 0, 32)
                    s8v = hv(rt[:, 256:512], 8, 0, 32)
                    ta = tp.tile([128, 256], bf16, tag="ta")
                    tb = tp.tile([128, 256], bf16, tag="tb")
                    tav = hv(ta[:], 8, 0, 32)
                    tbv = hv(tb[:], 8, 0, 32)
                    nc.vector.tensor_tensor(out=tav, in0=q1, in1=c8v, op=MUL)
                    nc.vector.tensor_tensor(out=tbv, in0=q2, in1=s8v, op=MUL)
                    nc.vector.tensor_tensor(out=hv(QKsb[:, 0:512], 8, 0, 32),
                                            in0=tav, in1=tbv, op=SUB)
                    tc_ = tp.tile([128, 256], bf16, tag="tc")
                    td_ = tp.tile([128, 256], bf16, tag="td")
                    tcv = hv(tc_[:], 8, 0, 32)
                    tdv = hv(td_[:], 8, 0, 32)
                    nc.vector.tensor_tensor(out=tcv, in0=q2, in1=c8v, op=MUL)
                    nc.vector.tensor_tensor(out=tdv, in0=q1, in1=s8v, op=MUL)
                    nc.vector.tensor_tensor(out=hv(QKsb[:, 0:512], 8, 32, 32),
                                            in0=tcv, in1=tdv, op=ADD)

                    # RoPE K
                    k1 = hv(Kb[:], 2, 0, 32)
                    k2 = hv(Kb[:], 2, 32, 32)
                    c2v = hv(rt[:, 0:64], 2, 0, 32)
                    s2v = hv(rt[:, 256:320], 2, 0, 32)
                    ka = tp.tile([128, 64], bf16, tag="ka")
                    kb = tp.tile([128, 64], bf16, tag="kb")
                    kav = hv(ka[:], 2, 0, 32)
                    kbv = hv(kb[:], 2, 0, 32)
                    nc.vector.tensor_tensor(out=kav, in0=k1, in1=c2v, op=MUL)
                    nc.vector.tensor_tensor(out=kbv, in0=k2, in1=s2v, op=MUL)
                    nc.vector.tensor_tensor(out=hv(QKsb[:, 512:640], 2, 0, 32),
                                            in0=kav, in1=kbv, op=SUB)
                    kc_ = tp.tile([128, 64], bf16, tag="kc")
                    kd_ = tp.tile([128, 64], bf16, tag="kd")
                    kcv = hv(kc_[:], 2, 0, 32)
                    kdv = hv(kd_[:], 2, 0, 32)
                    nc.vector.tensor_tensor(out=kcv, in0=k2, in1=c2v, op=MUL)
                    nc.vector.tensor_tensor(out=kdv, in0=k1, in1=s2v, op=MUL)
                    nc.vector.tensor_tensor(out=hv(QKsb[:, 512:640], 2, 32, 32),
                                            in0=kcv, in1=kdv, op=ADD)

                    # one merged xbar transpose per tc (Act ring)
                    ov = QTb[blk][:].rearrange("p (g t) -> p g t", g=5)[
                        :, :, tq * 128:(tq + 1) * 128]
                    nc.scalar.dma_start_transpose(ov, QKsb[:])
                    pump()
                    yield

            # ---------------- attention generator ------------------------
            def gen_attn(isc):
                njc = 4 * isc + 4
                for hp in range(4):
                    psOT = [psBp.tile([128, 512], fp32, tag="b", name=f"psOT{h2}")
                            for h2 in range(2)]

                    def emit_S(jc):
                        r = jc - 4 * isc
                        col0 = max(0, r * 128)
                        SA = sap.tile([128, 1024], bf16, tag="SA", name="SA")
                        psS = psSp.tile([128, 1024], fp32, tag="psS", name="psS")
                        jb, jr = jc // 4, jc % 4
                        for h2 in range(2):
                            prow = slice(64 * h2, 64 * h2 + 64)
                            nc.tensor.matmul(
                                psS[:, 512 * h2 + col0:512 * h2 + 512],
                                QTb[jb][prow, 2048 + jr * 128:2048 + jr * 128 + 128],
                                QTb[isc][prow, hp * 512 + col0:hp * 512 + 512],
                                start=True, stop=True,
                                tile_position=(64 * h2, 0),
                            )
                        if r < 0:
                            nc.scalar.activation(SA[:], psS[:], EXP, bias=nbias[:], scale=0.125)
                        else:
                            vi = psS[:].rearrange("p (h c) -> p h c", h=2)[:, :, col0:512]
                            vo = SA[:].rearrange("p (h c) -> p h c", h=2)[:, :, col0:512]
                            nc.scalar.activation(vo, vi, EXP, bias=nbias[:], scale=0.125)
                            mo = SA[:].rearrange("p (h c) -> p h c", h=2)[:, :, col0:col0 + 128]
                            mi = mask2[:].rearrange("p (h c) -> p h c", h=2)
                            nc.vector.tensor_tensor(out=mo, in0=mo, in1=mi, op=MUL)
                        return SA, col0

                    ready = emit_S(0)
                    for jc in range(njc):
                        SA, col0 = ready
                        if jc + 1 < njc:
                            ready = emit_S(jc + 1)
                        yield "jc"
                        for h2 in range(2):
                            nc.tensor.matmul(
                                psOT[h2][:, col0:512],
                                Vs[jc][:, 128 * h2:128 * h2 + 128],
                                SA[:, 512 * h2 + col0:512 * h2 + 512],
                                start=(jc == 0), stop=(jc == njc - 1),
                            )
                    # finalize: Z rows 64:128 of psOT came from the ones block
                    # of V; y_norm = psOT[0:64] * recip(Z) straight from PSUM.
                    for h2 in range(2):
                        # reciprocal_approx_fast only works at partition base 0:
                        # stage Z through SBUF first.
                        Zb = finp.tile([64, 512], fp32, tag="Zb", name="Zb")
                        nc.vector.tensor_copy(Zb[:], psOT[h2][64:128, :])
                        recC = finp.tile([64, 512], fp32, tag="recC", name="recC")
                        nc.vector.reciprocal_approx_fast(recC[:], Zb[:])
                        nc.vector.tensor_tensor(
                            out=yT[hp][64 * h2:64 * h2 + 64, isc * 512:(isc + 1) * 512],
                            in0=psOT[h2][0:64, :], in1=recC[:], op=MUL)
                    yield "posthp"

            # ---------------- stage C generator --------------------------
            def gen_C(isc):
                for t_i in range(4 * isc, 4 * isc + 4):
                    ts = slice(t_i * 128, (t_i + 1) * 128)
                    ot = osbp.tile([128, 2048], bf16, tag="ot")
                    for ec in range(4):
                        es = slice(ec * 512, (ec + 1) * 512)
                        psF = psBp.tile([128, 512], fp32, tag="b", name="psF")
                        for kc4 in range(4):
                            nc.tensor.matmul(
                                psF[:], yT[kc4][:, ts], wosAll[:, kc4 * T + ec * 512:
                                                               kc4 * T + (ec + 1) * 512],
                                start=(kc4 == 0), stop=(kc4 == 3),
                            )
                        nc.vector.tensor_copy(ot[:, es], psF[:])
                        if isc >= 2:
                            nc.sync.dma_start(out_d[ts, es], ot[:, es])
                        elif ec == 3:
                            nc.sync.dma_start(out_d[ts, :], ot[:])
                        yield

            # ---------------- driver --------------------------------------
            for _ in gen_A(0):
                pass
            psW2 = psQp.tile([128, 512], fp32, tag="psQ", name="psW2")
            for _ in range(30):
                nc.tensor.matmul(psW2[:, 0:128], iden[:], iden[:], start=True,
                                 stop=True, skip_group_check=True)
            for isc in range(4):
                ag = gen_A(isc + 1) if isc < 3 else None
                cg = gen_C(isc - 1) if isc > 0 else None
                # boundary cover: a few C chains keep the PE fed while the
                # last t-chunks' rope->transpose chains drain
                if cg is not None:
                    for _ in range(6):
                        if next(cg, "done") == "done":
                            cg = None
                            break
                n_jc = 4 * (4 * isc + 4)
                rate = 8.0 / (n_jc * 0.7)   # front-load A units
                acc = 0.0
                for ev in gen_attn(isc):
                    if ev == "jc" and ag is not None:
                        acc += rate
                        while acc >= 1.0:
                            if next(ag, "done") == "done":
                                ag = None
                                break
                            acc -= 1.0
                    elif ev == "posthp" and cg is not None:
                        for _ in range(3):
                            if next(cg, "done") == "done":
                                cg = None
                                break
                if ag is not None:
                    for _ in ag:
                        pass
                if cg is not None:
                    for _ in cg:
                        pass
            for _ in gen_C(3):
                pass

    nc.compile()
    return nc


def _host_prep(inputs):
    x = np.asarray(inputs["x"], np.float32)
    Wq = np.asarray(inputs["Wq"], np.float32)
    bq = np.asarray(inputs["bq"], np.float32)
    Wk = np.asarray(inputs["Wk"], np.float32)
    bk = np.asarray(inputs["bk"], np.float32)
    Wv = np.asarray(inputs["Wv"], np.float32)
    bv = np.asarray(inputs["bv"], np.float32)
    Wo = np.asarray(inputs["Wo"], np.float32)

    pos = np.arange(1, T + 1, dtype=np.float32)[:, None]
    freqs = 10000.0 ** (-(2.0 * np.arange(D // 2, dtype=np.float32)) / D)
    theta = pos * freqs
    cos_t = np.cos(theta).astype(np.float32)
    sin_t = np.sin(theta).astype(np.float32)
    ropeT = np.ascontiguousarray(np.concatenate(
        [np.tile(cos_t, (1, 8)), np.tile(sin_t, (1, 8))], axis=1)).astype(BF16)
    rope2 = np.ascontiguousarray(
        ropeT.reshape(NT, 128, 512).transpose(1, 0, 2).reshape(128, NT * 512))
    mask = (np.arange(128)[:, None] <= np.arange(128)[None, :]).astype(BF16)
    mask2 = np.ascontiguousarray(np.concatenate([mask, mask], axis=1))
    iden = np.eye(128, dtype=BF16)

    xT_aug = np.zeros((B, KAUG, T), np.float32)
    for b in range(B):
        xT_aug[b, :E] = x[b].T
        xT_aug[b, E] = 1.0
    xT_aug = xT_aug.astype(BF16)

    in_maps = []
    for c in range(8):
        b, p = c // 4, c % 4
        # head pair g = (global q-head 8p+g [kv 2p], 8p+4+g [kv 2p+1])
        qcols = []
        for g in range(4):
            qcols.extend(range(64 * (8 * p + g), 64 * (8 * p + g) + 64))
            qcols.extend(range(64 * (8 * p + 4 + g), 64 * (8 * p + 4 + g) + 64))
        qcols = np.array(qcols)
        wq_a = np.zeros((KAUG, 512), np.float32)
        wq_a[:E] = Wq[:, qcols]
        wq_a[E] = bq[qcols]
        wk_a = np.zeros((KAUG, 128), np.float32)
        wk_a[:E] = Wk[:, 128 * p:128 * (p + 1)]
        wk_a[E] = bk[128 * p:128 * (p + 1)]
        wv_a = np.zeros((KAUG, 128), np.float32)
        wv_a[:E] = Wv[:, 128 * p:128 * (p + 1)]
        wv_a[E] = bv[128 * p:128 * (p + 1)]
        wo_a = np.ascontiguousarray(Wo[qcols, :]).astype(BF16)
        wo2 = np.ascontiguousarray(
            wo_a.reshape(4, 128, T).transpose(1, 0, 2).reshape(128, 4 * T))
        in_maps.append({
            "xt": xT_aug[b],
            "wqkv": np.concatenate([wq_a, wk_a, wv_a], axis=1).astype(BF16),
            "wo2": wo2,
            "rope2": rope2,
            "mask2": mask2, "iden": iden,
        })
    return in_maps


def _run(inputs, trace=False):
    from concourse.bass_utils import run_bass_kernel_spmd

    if "nc" not in _cache:
        _cache["nc"] = _build_program()
    nc = _cache["nc"]
    in_maps = _host_prep(inputs)
    res = run_bass_kernel_spmd(nc, in_maps, core_ids=list(range(8)), trace=trace)
    bo = np.asarray(inputs["bo"], np.float32)
    out = np.zeros((B, T, E), np.float32)
    for b in range(B):
        acc = bo[None, :].repeat(T, 0).astype(np.float32)
        for c in range(4 * b, 4 * b + 4):
            acc = acc + res.results[c]["out"].astype(np.float32)
        out[b] = acc
    return out, res


def kernel(**inputs):
    out, _ = _run(inputs, trace=False)
    return out


# revision 18
# speedup vs baseline: 1.0371x; 1.0371x over previous
"""GQA forward on 8 Trainium2 NeuronCores — v4 (fused pipeline, lean DMA).

Sharding: core c -> batch b=c//4, kv-head pair p=c%4 (kv heads {2p,2p+1},
q heads 8p..8p+7). Each core computes a partial [T,E] output (its heads'
contribution through Wo rows); host sums the 4 partials per batch + bo.

v4 over v3: DMA instruction economy. v3's 80 per-head dma transposes cost
~1.2us of trigger descriptor-gen EACH on the issuing engine and their sem
waits head-of-line blocked the ScalarE exp queue, starving the PE for
~46us. v4 merges: one [128,640] xbar transpose per t-chunk into a packed
QTall tile (4 Q head-pair blocks + K block via 3D out AP) on the Act
ring; per-kc merged weight loads; 3-stage xt loads; single rope/wo loads;
one [128,2048] store per t-chunk. ScalarE carries only exp + 16 transpose
triggers; SP carries loads/stores.

Other structure (from v3):
- Stage A (QKV proj) software-pipelined WITH attention: projection chains
  for block k+1 and out-proj for k-1 interleave into attention block k.
- Q heads re-paired across kv groups (host Wq/Wo reorder) so the packed
  S matmul pair reads K^T from one tile without duplication.
- exp fused per chunk via strided [128,2,w] pair view; paired causal mask
  mul against a duplicated mask tile.
- Z replicated into psOT rows 64:128 by the 64-wide ones block in V;
  finalize = reciprocal + PSUM-sourced multiply into yT.
- PSUM: psS 2x[128,1024] + shared tag-"b" pool (psOT pairs / stage-C psF)
  + psQ + psKV = 8 banks.
"""
import sys
import numpy as np

sys.path.insert(0, "/opt/trn_rl_repo")

import ml_dtypes

BF16 = ml_dtypes.bfloat16

B, T, E = 2, 2048, 2048
HQ, HKV = 32, 8
D = 64
NT = T // 128          # 16 t-chunks
KC = 17                # augmented contraction chunks (2048 + bias row)
KAUG = KC * 128
NWARM = 200

_cache = {}


def _build_program():
    import concourse.bass as bass
    import concourse.tile as tile
    import concourse.mybir as mybir
    from concourse import bacc

    fp32 = mybir.dt.float32
    bf16 = mybir.dt.bfloat16
    MUL = mybir.AluOpType.mult
    ADD = mybir.AluOpType.add
    SUB = mybir.AluOpType.subtract
    EXP = mybir.ActivationFunctionType.Exp

    nc = bacc.Bacc("TRN2", target_bir_lowering=False, debug=False)

    xt_d = nc.dram_tensor("xt", [KAUG, T], bf16, kind="ExternalInput").ap()
    wqkv_d = nc.dram_tensor("wqkv", [KAUG, 768], bf16, kind="ExternalInput").ap()
    wo2_d = nc.dram_tensor("wo2", [128, 4 * T], bf16, kind="ExternalInput").ap()
    rope2_d = nc.dram_tensor("rope2", [128, NT * 512], bf16, kind="ExternalInput").ap()
    mask2_d = nc.dram_tensor("mask2", [128, 256], bf16, kind="ExternalInput").ap()
    iden_d = nc.dram_tensor("iden", [128, 128], bf16, kind="ExternalInput").ap()
    out_d = nc.dram_tensor("out", [T, E], bf16, kind="ExternalOutput").ap()

    def hv(ap, H, off, w):
        return ap.rearrange("p (h d) -> p h d", h=H)[:, :, off:off + w]

    with tile.TileContext(nc) as tc:
        with (
            tc.tile_pool(name="persist", bufs=1) as pp,
            tc.tile_pool(name="wpool", bufs=1) as wp,
            tc.tile_pool(name="xpool", bufs=1) as xp,
            tc.tile_pool(name="tmp", bufs=2) as tp,
            tc.tile_pool(name="qk", bufs=3) as qkp,
            tc.tile_pool(name="sa", bufs=3) as sap,
            tc.tile_pool(name="fin", bufs=2) as finp,
            tc.tile_pool(name="osb", bufs=2) as osbp,
            tc.tile_pool(name="psQ", bufs=1, space="PSUM") as psQp,
            tc.tile_pool(name="psKV", bufs=1, space="PSUM") as psKVp,
            tc.tile_pool(name="psS", bufs=2, space="PSUM") as psSp,
            tc.tile_pool(name="psB", bufs=2, space="PSUM") as psBp,
        ):
            iden = pp.tile([128, 128], bf16)
            nc.sync.dma_start(iden[:], iden_d[:])
            mask2 = pp.tile([128, 256], bf16)
            nc.sync.dma_start(mask2[:], mask2_d[:])
            rtsAll = xp.tile([128, NT * 512], bf16, tag="rts", name="rtsAll")
            nc.sync.dma_start(rtsAll[:, 0:2048], rope2_d[:, 0:2048])

            # per-isc-block transpose outputs: [128, 5*512]; within a block,
            # g=0..3 -> Q^T head pair g (512 cols each), block 4 -> K^T.
            # Separate tiles per isc block so the dep tracker's bounding-box
            # WAW does not chain transposes across blocks.
            QTb = [pp.tile([128, 5 * 512], bf16, tag=f"QTb{b}", name=f"QTb{b}")
                   for b in range(4)]
            yT = [pp.tile([128, T], bf16, tag=f"yT{g}", name=f"yT{g}") for g in range(4)]
            Vs = [pp.tile([128, 256], bf16, tag=f"V{j}", name=f"V{j}") for j in range(NT)]
            wqkvAll = wp.tile([128, KC * 768], bf16, tag="wqkv", name="wqkvAll")
            wosAll = wp.tile([128, 4 * T], bf16, tag="wos", name="wosAll")
            xtAll = xp.tile([128, KC * T], bf16, tag="xtAll", name="xtAll")

            # ---- prologue loads (SP ring) as a handful of mega-DMAs with
            # 3D APs: few DMA instructions keep the sem-slot-reuse guards on
            # later transposes/stores pointing at long-completed transfers.
            def xt_mega(kcs, cs):
                ov = xtAll[:].rearrange("p (kc t) -> p kc t", kc=KC)[:, kcs, cs]
                iv = xt_d[:, cs].rearrange("(kc p) t -> p kc t", p=128)[:, kcs, :]
                nc.sync.dma_start(ov, iv)

            def wqkv_mega(kcs):
                ov = wqkvAll[:].rearrange("p (kc w) -> p kc w", kc=KC)[:, kcs, :]
                iv = wqkv_d[:].rearrange("(kc p) w -> p kc w", p=128)[:, kcs, :]
                nc.sync.dma_start(ov, iv)

            wqkv_mega(slice(0, 9))
            xt_mega(slice(0, 9), slice(0, 512))
            wqkv_mega(slice(9, KC))
            xt_mega(slice(9, KC), slice(0, 512))
            xt_mega(slice(0, 9), slice(512, 1024))
            xt_mega(slice(9, KC), slice(512, 1024))
            nc.sync.dma_start(rtsAll[:, 2048:8192], rope2_d[:, 2048:8192])
            nc.sync.dma_start(wosAll[:], wo2_d[:])
            xt_mega(slice(0, 9), slice(1024, 2048))
            xt_mega(slice(9, KC), slice(1024, 2048))



            for j in range(NT):
                nc.gpsimd.memset(Vs[j][:, 64:128], 1.0)
                nc.gpsimd.memset(Vs[j][:, 192:256], 1.0)

            nbias = pp.tile([128, 1], fp32, tag="nbias", name="nbias")
            nc.gpsimd.memset(nbias[:], -4.0)

            # PE warm-up riding the initial DMA gate (uses the psQ bank)
            psW = psQp.tile([128, 512], fp32, tag="psQ", name="psW")
            for _ in range(NWARM):
                nc.tensor.matmul(psW[:, 0:128], iden[:], iden[:], start=True,
                                 stop=True, skip_group_check=True)

            # ---------------- stage A generator --------------------------
            def gen_A(blk):
                def pump():
                    pass

                for tq in range(4):
                    t_i = 4 * blk + tq
                    ts = slice(t_i * 128, (t_i + 1) * 128)
                    xsl = slice(blk * 512 + tq * 128, blk * 512 + (tq + 1) * 128)
                    psQ = psQp.tile([128, 512], fp32, tag="psQ", name="psQ")
                    for kc in range(KC):
                        nc.tensor.matmul(psQ[:], xtAll[:, kc * T + xsl.start:kc * T + xsl.stop],
                                         wqkvAll[:, kc * 768:kc * 768 + 512],
                                         start=(kc == 0), stop=(kc == KC - 1))
                    pump()
                    yield
                    psKV = psKVp.tile([128, 256], fp32, tag="psKV", name="psKV")
                    for kc in range(KC):
                        nc.tensor.matmul(psKV[:], xtAll[:, kc * T + xsl.start:kc * T + xsl.stop],
                                         wqkvAll[:, kc * 768 + 512:kc * 768 + 768],
                                         start=(kc == 0), stop=(kc == KC - 1))
                    # casts + V copy (DVE; also frees psQ/psKV for next tc)
                    Qb = tp.tile([128, 512], bf16, tag="Qb")
                    nc.vector.tensor_copy(Qb[:], psQ[:])
                    Kb = tp.tile([128, 128], bf16, tag="Kb")
                    nc.vector.tensor_copy(Kb[:], psKV[:, 0:128])
                    vo = Vs[t_i][:].rearrange("p (kv c) -> p kv c", kv=2)[:, :, 0:64]
                    vi = psKV[:, 128:256].rearrange("p (kv c) -> p kv c", kv=2)
                    nc.vector.tensor_copy(vo, vi)

                    rt = rtsAll[:, t_i * 512:(t_i + 1) * 512]
                    QKsb = qkp.tile([128, 640], bf16, tag="QKsb")
                    # RoPE Q (DVE bf16): y1' = y1*c - y2*s ; y2' = y2*c + y1*s
                    q1 = hv(Qb[:], 8, 0, 32)
                    q2 = hv(Qb[:], 8, 32, 32)
                    c8v = hv(rt[:, 0:256], 8, 0, 32)
                    s8v = hv(rt[:, 256:512], 8, 0, 32)
                    ta = tp.tile([128, 256], bf16, tag="ta")
                    tb = tp.tile([128, 256], bf16, tag="tb")
                    tav = hv(ta[:], 8, 0, 32)
                    tbv = hv(tb[:], 8, 0, 32)
                    nc.vector.tensor_tensor(out=tav, in0=q1, in1=c8v, op=MUL)
                    nc.vector.tensor_tensor(out=tbv, in0=q2, in1=s8v, op=MUL)
                    nc.vector.tensor_tensor(out=hv(QKsb[:, 0:512], 8, 0, 32),
                                            in0=tav, in1=tbv, op=SUB)
                    tc_ = tp.tile([128, 256], bf16, tag="tc")
                    td_ = tp.tile([128, 256], bf16, tag="td")
                    tcv = hv(tc_[:], 8, 0, 32)
                    tdv = hv(td_[:], 8, 0, 32)
                    nc.vector.tensor_tensor(out=tcv, in0=q2, in1=c8v, op=MUL)
                    nc.vector.tensor_tensor(out=tdv, in0=q1, in1=s8v, op=MUL)
                    nc.vector.tensor_tensor(out=hv(QKsb[:, 0:512], 8, 32, 32),
                                            in0=tcv, in1=tdv, op=ADD)

                    # RoPE K
                    k1 = hv(Kb[:], 2, 0, 32)
                    k2 = hv(Kb[:], 2, 32, 32)
                    c2v = hv(rt[:, 0:64], 2, 0, 32)
                    s2v = hv(rt[:, 256:320], 2, 0, 32)
                    ka = tp.tile([128, 64], bf16, tag="ka")
                    kb = tp.tile([128, 64], bf16, tag="kb")
                    kav = hv(ka[:], 2, 0, 32)
                    kbv = hv(kb[:], 2, 0, 32)
                    nc.vector.tensor_tensor(out=kav, in0=k1, in1=c2v, op=MUL)
                    nc.vector.tensor_tensor(out=kbv, in0=k2, in1=s2v, op=MUL)
                    nc.vector.tensor_tensor(out=hv(QKsb[:, 512:640], 2, 0, 32),
                                            in0=kav, in1=kbv, op=SUB)
                    kc_ = tp.tile([128, 64], bf16, tag="kc")
                    kd_ = tp.tile([128, 64], bf16, tag="kd")
                    kcv = hv(kc_[:], 2, 0, 32)
                    kdv = hv(kd_[:], 2, 0, 32)
                    nc.vector.tensor_tensor(out=kcv, in0=k2, in1=c2v, op=MUL)
                    nc.vector.tensor_tensor(out=kdv, in0=k1, in1=s2v, op=MUL)
                    nc.vector.tensor_tensor(out=hv(QKsb[:, 512:640], 2, 32, 32),
                                            in0=kcv, in1=kdv, op=ADD)

                    # PE transposes into a psS-tag PSUM slot, one DVE copy out
                    psT = psSp.tile([128, 640], bf16, tag="psS", name="psT")
                    for g in range(5):
                        nc.tensor.transpose(psT[:, g * 128:(g + 1) * 128],
                                            QKsb[:, g * 128:(g + 1) * 128], iden[:])
                    ov = QTb[blk][:].rearrange("p (g t) -> p g t", g=5)[
                        :, :, tq * 128:(tq + 1) * 128]
                    iv = psT[:].rearrange("p (g t) -> p g t", g=5)
                    nc.vector.tensor_copy(ov, iv)
                    pump()
                    yield

            # ---------------- attention generator ------------------------
            def gen_attn(isc):
                njc = 4 * isc + 4
                for hp in range(4):
                    psOT = [psBp.tile([128, 512], fp32, tag="b", name=f"psOT{h2}")
                            for h2 in range(2)]

                    def emit_S(jc):
                        r = jc - 4 * isc
                        col0 = max(0, r * 128)
                        SA = sap.tile([128, 1024], bf16, tag="SA", name="SA")
                        psS = psSp.tile([128, 1024], fp32, tag="psS", name="psS")
                        jb, jr = jc // 4, jc % 4
                        for h2 in range(2):
                            prow = slice(64 * h2, 64 * h2 + 64)
                            nc.tensor.matmul(
                                psS[:, 512 * h2 + col0:512 * h2 + 512],
                                QTb[jb][prow, 2048 + jr * 128:2048 + jr * 128 + 128],
                                QTb[isc][prow, hp * 512 + col0:hp * 512 + 512],
                                start=True, stop=True,
                                tile_position=(64 * h2, 0),
                            )
                        if r < 0:
                            nc.scalar.activation(SA[:], psS[:], EXP, bias=nbias[:], scale=0.125)
                        else:
                            vi = psS[:].rearrange("p (h c) -> p h c", h=2)[:, :, col0:512]
                            vo = SA[:].rearrange("p (h c) -> p h c", h=2)[:, :, col0:512]
                            nc.scalar.activation(vo, vi, EXP, bias=nbias[:], scale=0.125)
                            mo = SA[:].rearrange("p (h c) -> p h c", h=2)[:, :, col0:col0 + 128]
                            mi = mask2[:].rearrange("p (h c) -> p h c", h=2)
                            nc.vector.tensor_tensor(out=mo, in0=mo, in1=mi, op=MUL)
                        return SA, col0

                    ready = emit_S(0)
                    for jc in range(njc):
                        SA, col0 = ready
                        if jc + 1 < njc:
                            ready = emit_S(jc + 1)
                        yield "jc"
                        for h2 in range(2):
                            nc.tensor.matmul(
                                psOT[h2][:, col0:512],
                                Vs[jc][:, 128 * h2:128 * h2 + 128],
                                SA[:, 512 * h2 + col0:512 * h2 + 512],
                                start=(jc == 0), stop=(jc == njc - 1),
                            )
                    # finalize: Z rows 64:128 of psOT came from the ones block
                    # of V; y_norm = psOT[0:64] * recip(Z) straight from PSUM.
                    for h2 in range(2):
                        # reciprocal_approx_fast only works at partition base 0:
                        # stage Z through SBUF first.
                        Zb = finp.tile([64, 512], fp32, tag="Zb", name="Zb")
                        nc.vector.tensor_copy(Zb[:], psOT[h2][64:128, :])
                        recC = finp.tile([64, 512], fp32, tag="recC", name="recC")
                        nc.vector.reciprocal_approx_fast(recC[:], Zb[:])
                        nc.vector.tensor_tensor(
                            out=yT[hp][64 * h2:64 * h2 + 64, isc * 512:(isc + 1) * 512],
                            in0=psOT[h2][0:64, :], in1=recC[:], op=MUL)
                    yield "posthp"

            # ---------------- stage C generator --------------------------
            def gen_C(isc):
                for t_i in range(4 * isc, 4 * isc + 4):
                    ts = slice(t_i * 128, (t_i + 1) * 128)
                    ot = osbp.tile([128, 2048], bf16, tag="ot")
                    for ec in range(4):
                        es = slice(ec * 512, (ec + 1) * 512)
                        psF = psBp.tile([128, 512], fp32, tag="b", name="psF")
                        for kc4 in range(4):
                            nc.tensor.matmul(
                                psF[:], yT[kc4][:, ts], wosAll[:, kc4 * T + ec * 512:
                                                               kc4 * T + (ec + 1) * 512],
                                start=(kc4 == 0), stop=(kc4 == 3),
                            )
                        nc.vector.tensor_copy(ot[:, es], psF[:])
                        if isc >= 2:
                            nc.sync.dma_start(out_d[ts, es], ot[:, es])
                        elif ec == 3:
                            nc.sync.dma_start(out_d[ts, :], ot[:])
                        yield

            # ---------------- driver --------------------------------------
            for _ in gen_A(0):
                pass
            psW2 = psQp.tile([128, 512], fp32, tag="psQ", name="psW2")
            for _ in range(30):
                nc.tensor.matmul(psW2[:, 0:128], iden[:], iden[:], start=True,
                                 stop=True, skip_group_check=True)
            for isc in range(4):
                ag = gen_A(isc + 1) if isc < 3 else None
                cg = gen_C(isc - 1) if isc > 0 else None
                # boundary cover: a few C chains keep the PE fed while the
                # last t-chunks' rope->transpose chains drain
                if cg is not None:
                    for _ in range(6):
                        if next(cg, "done") == "done":
                            cg = None
                            break
                n_jc = 4 * (4 * isc + 4)
                rate = 8.0 / (n_jc * 0.7)   # front-load A units
                acc = 0.0
                for ev in gen_attn(isc):
                    if ev == "jc" and ag is not None:
                        acc += rate
                        while acc >= 1.0:
                            if next(ag, "done") == "done":
                                ag = None
                                break
                            acc -= 1.0
                    elif ev == "posthp" and cg is not None:
                        for _ in range(3):
                            if next(cg, "done") == "done":
                                cg = None
                                break
                if ag is not None:
                    for _ in ag:
                        pass
                if cg is not None:
                    for _ in cg:
                        pass
            for _ in gen_C(3):
                pass

    nc.compile()
    return nc


def _host_prep(inputs):
    x = np.asarray(inputs["x"], np.float32)
    Wq = np.asarray(inputs["Wq"], np.float32)
    bq = np.asarray(inputs["bq"], np.float32)
    Wk = np.asarray(inputs["Wk"], np.float32)
    bk = np.asarray(inputs["bk"], np.float32)
    Wv = np.asarray(inputs["Wv"], np.float32)
    bv = np.asarray(inputs["bv"], np.float32)
    Wo = np.asarray(inputs["Wo"], np.float32)

    pos = np.arange(1, T + 1, dtype=np.float32)[:, None]
    freqs = 10000.0 ** (-(2.0 * np.arange(D // 2, dtype=np.float32)) / D)
    theta = pos * freqs
    cos_t = np.cos(theta).astype(np.float32)
    sin_t = np.sin(theta).astype(np.float32)
    ropeT = np.ascontiguousarray(np.concatenate(
        [np.tile(cos_t, (1, 8)), np.tile(sin_t, (1, 8))], axis=1)).astype(BF16)
    rope2 = np.ascontiguousarray(
        ropeT.reshape(NT, 128, 512).transpose(1, 0, 2).reshape(128, NT * 512))
    mask = (np.arange(128)[:, None] <= np.arange(128)[None, :]).astype(BF16)
    mask2 = np.ascontiguousarray(np.concatenate([mask, mask], axis=1))
    iden = np.eye(128, dtype=BF16)

    xT_aug = np.zeros((B, KAUG, T), np.float32)
    for b in range(B):
        xT_aug[b, :E] = x[b].T
        xT_aug[b, E] = 1.0
    xT_aug = xT_aug.astype(BF16)

    in_maps = []
    for c in range(8):
        b, p = c // 4, c % 4
        # head pair g = (global q-head 8p+g [kv 2p], 8p+4+g [kv 2p+1])
        qcols = []
        for g in range(4):
            qcols.extend(range(64 * (8 * p + g), 64 * (8 * p + g) + 64))
            qcols.extend(range(64 * (8 * p + 4 + g), 64 * (8 * p + 4 + g) + 64))
        qcols = np.array(qcols)
        wq_a = np.zeros((KAUG, 512), np.float32)
        wq_a[:E] = Wq[:, qcols]
        wq_a[E] = bq[qcols]
        wk_a = np.zeros((KAUG, 128), np.float32)
        wk_a[:E] = Wk[:, 128 * p:128 * (p + 1)]
        wk_a[E] = bk[128 * p:128 * (p + 1)]
        wv_a = np.zeros((KAUG, 128), np.float32)
        wv_a[:E] = Wv[:, 128 * p:128 * (p + 1)]
        wv_a[E] = bv[128 * p:128 * (p + 1)]
        wo_a = np.ascontiguousarray(Wo[qcols, :]).astype(BF16)
        wo2 = np.ascontiguousarray(
            wo_a.reshape(4, 128, T).transpose(1, 0, 2).reshape(128, 4 * T))
        in_maps.append({
            "xt": xT_aug[b],
            "wqkv": np.concatenate([wq_a, wk_a, wv_a], axis=1).astype(BF16),
            "wo2": wo2,
            "rope2": rope2,
            "mask2": mask2, "iden": iden,
        })
    return in_maps


def _run(inputs, trace=False):
    from concourse.bass_utils import run_bass_kernel_spmd

    if "nc" not in _cache:
        _cache["nc"] = _build_program()
    nc = _cache["nc"]
    in_maps = _host_prep(inputs)
    res = run_bass_kernel_spmd(nc, in_maps, core_ids=list(range(8)), trace=trace)
    bo = np.asarray(inputs["bo"], np.float32)
    out = np.zeros((B, T, E), np.float32)
    for b in range(B):
        acc = bo[None, :].repeat(T, 0).astype(np.float32)
        for c in range(4 * b, 4 * b + 4):
            acc = acc + res.results[c]["out"].astype(np.float32)
        out[b] = acc
    return out, res


def kernel(**inputs):
    out, _ = _run(inputs, trace=False)
    return out
